# revision 1
# baseline (speedup 1.0000x reference)
"""Causal self-attention (B=2, S=2048, D=1024, H=16) on 8 TRN2 NeuronCores.

Sharding: core c -> batch b = c//4, head group g = c%4 (heads 4g..4g+4,
i.e. 256 of the 1024 projection dims). No collectives: each core emits a
transposed partial output out.T = (ans_local @ Wo_cols.T).T of shape
[1024, 2048]; the host transposes and sums the 4 partials per batch.

Device kernel (per core, bf16 matmuls with f32 PSUM accumulation):
  1. QKV projections from pre-transposed x.T/W.T tiles -> Q.T, K.T
     ([head_dim, seq] layout, head pairs stacked on 128 partitions) and
     V ([seq, 128] per k-tile: cols 0-63 = head values, 64-127 = ones).
  2. Attention per head pair in the transposed layout: S.T[k, q] for both
     heads row-packed into one [128, 1024] PSUM tile (keeps all 128 PE
     rows active -> HAM stays unthrottled), causal mask add on diagonal
     blocks, one exp per k-tile on ScalarE (scale=1/8 folded in), then
     O.T[128, q] = V_aug^T-free matmul (lhsT=V_aug, rhs=P.T). Rows
     64-127 of O.T are the softmax denominators (replicated).
  3. Normalization per q-chunk: collect the 4 units' denominator rows
     into [4, 512], one cheap reciprocal, partition-broadcast each row
     via SBUF->SBUF DMA, multiply into ans.T (bf16).
  4. Output projection: out.T[n, q] = Wo.T^T @ ans.T, streamed to DRAM.
"""
import sys

if "/opt/trn_rl_repo" not in sys.path:
    sys.path.insert(0, "/opt/trn_rl_repo")

import numpy as np
import ml_dtypes

import concourse.bacc as bacc
import concourse.tile as tile
from concourse import mybir
from concourse.bass_utils import run_bass_kernel_spmd

N_CORES = 8
B, S, D, H = 2, 2048, 1024, 16
HD = D // H          # 64
HEADS_PER_CORE = 4   # 2 pairs
MLOC = HEADS_PER_CORE * HD  # 256 local projection dims per core
QC = 512             # q chunk width
NQC = S // QC        # 4
NKT = S // 128       # 16 k tiles of 128
KT_PER_QC = QC // 128  # 4

BF16 = mybir.dt.bfloat16
F32 = mybir.dt.float32
AF = mybir.ActivationFunctionType

_CACHED_NC = None


def _build_nc():
    nc = bacc.Bacc("TRN2", target_bir_lowering=False, debug=False,
                   enable_asserts=False, num_devices=N_CORES)

    xt_d = nc.dram_tensor("xt", [D, S], BF16, kind="ExternalInput").ap()
    wqt_d = nc.dram_tensor("wqt", [D, MLOC], BF16, kind="ExternalInput").ap()
    wkt_d = nc.dram_tensor("wkt", [D, MLOC], BF16, kind="ExternalInput").ap()
    wvt_d = nc.dram_tensor("wvt", [D, MLOC], BF16, kind="ExternalInput").ap()
    wot_d = nc.dram_tensor("wot", [MLOC, D], BF16, kind="ExternalInput").ap()
    mask_d = nc.dram_tensor("mask", [128, 128], BF16, kind="ExternalInput").ap()
    ident_d = nc.dram_tensor("ident", [128, 128], BF16, kind="ExternalInput").ap()
    ind_d = nc.dram_tensor("ind", [97, 256], BF16, kind="ExternalInput").ap()
    out_d = nc.dram_tensor("out", [D, S], BF16, kind="ExternalOutput").ap()

    with tile.TileContext(nc) as tc:
        with tc.tile_pool(name="const", bufs=1) as cpool, \
             tc.tile_pool(name="qkv_sb", bufs=1) as qkvpool, \
             tc.tile_pool(name="pt", bufs=4) as ptpool, \
             tc.tile_pool(name="norm", bufs=2) as normpool, \
             tc.tile_pool(name="ostage", bufs=8) as opool, \
             tc.tile_pool(name="au", bufs=4) as aupool, \
             tc.tile_pool(name="ps_big", bufs=2, space="PSUM") as psb, \
             tc.tile_pool(name="ps_ot", bufs=2, space="PSUM") as psot, \
             tc.tile_pool(name="ps_fill", bufs=2, space="PSUM") as psf:

            # ---- constants / inputs ----
            # x.T, d-major tiles, loaded in (dt, qc) chunks so the first
            # QKV matmuls can start after ~1MB instead of the full 4MB.
            xt = cpool.tile([128, 8, S], BF16)
            wqt = cpool.tile([128, 8, MLOC], BF16)
            wkt = cpool.tile([128, 8, MLOC], BF16)
            wvt = cpool.tile([128, 8, MLOC], BF16)
            wqt_r = wqt_d.rearrange("(t p) m -> p t m", p=128)
            wkt_r = wkt_d.rearrange("(t p) m -> p t m", p=128)
            wvt_r = wvt_d.rearrange("(t p) m -> p t m", p=128)
            xt_r = xt_d.rearrange("(t p) s -> p t s", p=128)
            nc.sync.dma_start(wqt[:], wqt_r)
            nc.scalar.dma_start(wkt[:], wkt_r)
            nc.sync.dma_start(xt[:, :, 0:QC], xt_r[:, :, 0:QC])
            nc.scalar.dma_start(wvt[:], wvt_r)
            nc.sync.dma_start(xt[:, :, QC:2 * QC], xt_r[:, :, QC:2 * QC])
            nc.scalar.dma_start(xt[:, :, 2 * QC:3 * QC],
                                xt_r[:, :, 2 * QC:3 * QC])
            nc.sync.dma_start(xt[:, :, 3 * QC:4 * QC],
                              xt_r[:, :, 3 * QC:4 * QC])
            wot = cpool.tile([128, 2, D], BF16)
            nc.scalar.dma_start(wot[:], wot_d.rearrange("(t p) m -> p t m",
                                                        p=128))
            mask = cpool.tile([128, 128], BF16)
            nc.sync.dma_start(mask[:], mask_d[:])
            ident = cpool.tile([128, 128], BF16)
            nc.sync.dma_start(ident[:], ident_d[:])
            # indicator rows: ind[:, 64u:64u+64] is one-hot row u -> used as
            # matmul lhsT to broadcast row u of a [4, N] tile to 64 partitions
            ind = cpool.tile([97, 256], BF16)
            nc.sync.dma_start(ind[:], ind_d[:])
            # denominator collector rows live at partitions 0/32/64/96
            # (compute-engine APs need 32-aligned partition bases)
            srows = cpool.tile([97, QC], F32)
            nc.vector.memset(srows[:], 1.0)
            rq = cpool.tile([97, QC], F32)
            rq16 = cpool.tile([97, QC], BF16)

            # ---- QKV projections ----
            # QT/KT: [m-local(2 heads)=128, S] per pair.
            # V: [s=128, kt, head, 128]: cols 0-63 values, 64-127 ones.
            QT = [qkvpool.tile([128, S], BF16, tag=f"qt{p}", name=f"qt{p}")
                  for p in range(2)]
            KT = [qkvpool.tile([128, S], BF16, tag=f"kt{p}", name=f"ktile{p}")
                  for p in range(2)]
            V = qkvpool.tile([128, NKT, HEADS_PER_CORE, 128], BF16)
            ansT = [qkvpool.tile([128, S], BF16, tag=f"at{p}", name=f"at{p}")
                    for p in range(2)]

            nc.vector.memset(V[:, :, :, HD:], 1.0)

            class _SC:
                def tensor_copy(self, out, in_):
                    return nc.scalar.copy(out, in_)
            sceng = _SC()

            # ---- filler machinery: QKV projection work is emitted in small
            # increments between attention k-tiles so the PE stream stays
            # dense while ScalarE runs the exps. PSUM: ps_fill pool.
            def qk_gen(p, qc, ceng=None):
                ps_q = psf.tile([128, QC], F32, tag="fill", name="ps_q")
                ps_k = psf.tile([128, QC], F32, tag="fill", name="ps_k")
                for dt in range(8):
                    nc.tensor.matmul(
                        ps_q[:], wqt[:, dt, 128 * p:128 * (p + 1)],
                        xt[:, dt, QC * qc:QC * (qc + 1)],
                        start=(dt == 0), stop=(dt == 7))
                    nc.tensor.matmul(
                        ps_k[:], wkt[:, dt, 128 * p:128 * (p + 1)],
                        xt[:, dt, QC * qc:QC * (qc + 1)],
                        start=(dt == 0), stop=(dt == 7))
                    yield
                (ceng or nc.vector).tensor_copy(
                    QT[p][:, QC * qc:QC * (qc + 1)], ps_q[:])
                (ceng or nc.vector).tensor_copy(
                    KT[p][:, QC * qc:QC * (qc + 1)], ps_k[:])

            def v_gen(st, ceng=None):
                ps_v = psf.tile([128, QC], F32, tag="fill", name="ps_v")
                for dt in range(8):
                    nc.tensor.matmul(
                        ps_v[:, 0:MLOC], xt[:, dt, 128 * st:128 * (st + 1)],
                        wvt[:, dt, :], start=(dt == 0), stop=(dt == 7))
                    if dt % 2 == 1:
                        yield
                (ceng or nc.vector).tensor_copy(
                    V[:, st, :, 0:HD],
                    ps_v[:, 0:MLOC].rearrange("p (h c) -> p h c",
                                              h=HEADS_PER_CORE))

            # stream of filler units with labels for dependency gating
            fill_units = []
            for st in range(4, 8):
                fill_units.append((("v", st), v_gen(st, sceng)))
            fill_units.append((("qk", 0, 1), qk_gen(0, 1, sceng)))
            for st in range(8, 12):
                fill_units.append((("v", st), v_gen(st, sceng)))
            fill_units.append((("qk", 0, 2), qk_gen(0, 2, sceng)))
            for st in range(12, 16):
                fill_units.append((("v", st), v_gen(st, sceng)))
            fill_units.append((("qk", 0, 3), qk_gen(0, 3, sceng)))
            for qc in range(NQC):
                fill_units.append((("qk", 1, qc), qk_gen(1, qc)))
            done_units = set()

            def pump(n):
                k = 0
                while k < n and fill_units:
                    label, gen = fill_units[0]
                    try:
                        next(gen)
                        k += 1
                    except StopIteration:
                        done_units.add(label)
                        fill_units.pop(0)

            def require(labels):
                for lab in labels:
                    while fill_units and lab not in done_units:
                        cur_lab, gen = fill_units[0]
                        for _ in gen:
                            pass
                        done_units.add(cur_lab)
                        fill_units.pop(0)
                        if cur_lab == lab:
                            break

            def wo_proj(qc):
                for nt in range(8):
                    pool = psf if nt % 2 == 0 else psot
                    po = pool.tile([128, QC], F32,
                                   tag="fill" if nt % 2 == 0 else "ot",
                                   name="po")
                    for mt in range(2):
                        nc.tensor.matmul(
                            po[:, 0:QC],
                            wot[:, mt, 128 * nt:128 * (nt + 1)],
                            ansT[mt][:, QC * qc:QC * (qc + 1)],
                            start=(mt == 0), stop=(mt == 1))
                    ob = opool.tile([128, QC], BF16, tag="ob", name="ob")
                    if nt % 2 == 0:
                        nc.vector.tensor_copy(ob[:], po[:, 0:QC])
                    else:
                        nc.scalar.copy(ob[:], po[:, 0:QC])
                    eng = nc.sync if nt % 2 == 0 else nc.scalar
                    eng.dma_start(
                        out_d[128 * nt:128 * (nt + 1), QC * qc:QC * (qc + 1)],
                        ob[:])

            def make_norm(p, qc, au_a, au_b):
                def norm():
                    rbase = 64 * p
                    nc.vector.reciprocal(rq[rbase:rbase + 33, :],
                                         srows[rbase:rbase + 33, :])
                    nc.vector.tensor_copy(rq16[rbase:rbase + 33, :],
                                          rq[rbase:rbase + 33, :])
                    for h in range(2):
                        u = 2 * p + h
                        bc = psb.tile([64, QC], F32, tag="big", name=f"bc{u}")
                        nc.tensor.matmul(bc[:],
                                         ind[rbase:rbase + 33,
                                             64 * u:64 * (u + 1)],
                                         rq16[rbase:rbase + 33, :],
                                         start=True, stop=True)
                        bcs = normpool.tile([64, QC], BF16, tag=f"bcs{u}",
                                            name=f"bcs{u}")
                        nc.vector.tensor_copy(bcs[:], bc[:])
                        nc.vector.tensor_mul(
                            ansT[p][64 * h:64 * (h + 1),
                                    QC * qc:QC * (qc + 1)],
                            (au_a if h == 0 else au_b)[:], bcs[:])
                return norm

            deferred = []

            def attn(p, qc):
                nkt = KT_PER_QC * (qc + 1)
                ot_a = psot.tile([128, QC], F32, tag="ot", name="ot_a")
                ot_b = psot.tile([128, QC], F32, tag="ot", name="ot_b")
                for kt in range(nkt):
                    r = kt - KT_PER_QC * qc
                    col0 = 128 * r if r >= 0 else 0
                    stp = psb.tile([128, 2 * QC], F32, tag="big", name="stp")
                    pt = ptpool.tile([128, 2 * QC], BF16, tag="pt", name="pt")
                    diag = r >= 0
                    nc.tensor.matmul(
                        stp[:, col0:QC],
                        KT[p][0:64, 128 * kt:128 * (kt + 1)],
                        QT[p][0:64, QC * qc + col0:QC * (qc + 1)],
                        start=True, stop=not diag)
                    if diag:
                        nc.tensor.matmul(
                            stp[:, col0:col0 + 128], ident[:], mask[:],
                            start=False, stop=True)
                    nc.tensor.matmul(
                        stp[:, QC + col0:2 * QC],
                        KT[p][64:128, 128 * kt:128 * (kt + 1)],
                        QT[p][64:128, QC * qc + col0:QC * (qc + 1)],
                        start=True, stop=not diag)
                    if diag:
                        nc.tensor.matmul(
                            stp[:, QC + col0:QC + col0 + 128], ident[:],
                            mask[:], start=False, stop=True)
                    if r > 0:
                        sv = stp[:].rearrange("p (h q) -> p h q",
                                              h=2)[:, :, col0:]
                        pv = pt[:].rearrange("p (h q) -> p h q",
                                             h=2)[:, :, col0:]
                        nc.scalar.activation(pv, sv, AF.Exp, scale=0.125)
                    else:
                        nc.scalar.activation(pt[:], stp[:], AF.Exp,
                                             scale=0.125)
                    nc.tensor.matmul(
                        ot_a[:, col0:QC], V[:, kt, 2 * p, :],
                        pt[:, col0:QC],
                        start=(kt == 0), stop=(kt == nkt - 1))
                    nc.tensor.matmul(
                        ot_b[:, col0:QC], V[:, kt, 2 * p + 1, :],
                        pt[:, QC + col0:2 * QC],
                        start=(kt == 0), stop=(kt == nkt - 1))
                    if kt == 1:
                        while deferred:
                            deferred.pop(0)()
                    pump(2)
                # unit end: extract denominators + unnormalized O.T to SBUF
                # so the PSUM accumulators free immediately; the reciprocal/
                # broadcast/multiply chain is deferred into the next unit.
                rbase = 64 * p
                au_a = aupool.tile([64, QC], BF16, tag="au", name="au_a")
                au_b = aupool.tile([64, QC], BF16, tag="au", name="au_b")
                nc.vector.tensor_copy(srows[rbase:rbase + 1, :],
                                      ot_a[64:65, :])
                nc.vector.tensor_copy(au_a[:], ot_a[0:64, :])
                nc.vector.tensor_copy(srows[rbase + 32:rbase + 33, :],
                                      ot_b[64:65, :])
                nc.vector.tensor_copy(au_b[:], ot_b[0:64, :])
                return make_norm(p, qc, au_a, au_b)

            # pre-work for the first attention unit
            pre_q = qk_gen(0, 0)
            for _ in pre_q:
                pass
            for st in range(4):
                for _ in v_gen(st):
                    pass

            reqs = {
                (0, 1): [("qk", 0, 1), ("v", 7)],
                (0, 2): [("qk", 0, 2), ("v", 11)],
                (0, 3): [("qk", 0, 3), ("v", 15)],
                (1, 0): [("qk", 1, 0), ("v", 15)],
                (1, 1): [("qk", 1, 1)],
                (1, 2): [("qk", 1, 2)],
                (1, 3): [("qk", 1, 3)],
            }
            for p in range(2):
                for qc in range(NQC):
                    require(reqs.get((p, qc), []))
                    deferred.append(attn(p, qc))
            while deferred:
                deferred.pop(0)()
            for qc in range(NQC):
                wo_proj(qc)

    nc.compile()
    return nc


def _get_nc():
    global _CACHED_NC
    if _CACHED_NC is None:
        _CACHED_NC = _build_nc()
    return _CACHED_NC


def _make_in_maps(x, Wq, Wk, Wv, Wo):
    bf16 = ml_dtypes.bfloat16
    mask = np.where(np.arange(128)[:, None] > np.arange(128)[None, :],
                    np.float32(-3e8), np.float32(0.0)).astype(bf16)
    identm = np.eye(128, dtype=bf16)
    indm = np.zeros((97, 256), dtype=bf16)
    for u in range(4):
        indm[32 * u, 64 * u:64 * (u + 1)] = 1.0
    in_maps = []
    for c in range(N_CORES):
        b, g = divmod(c, 4)
        ms = slice(MLOC * g, MLOC * (g + 1))
        in_maps.append({
            "xt": np.ascontiguousarray(x[b].T).astype(bf16),
            "wqt": np.ascontiguousarray(Wq[ms, :].T).astype(bf16),
            "wkt": np.ascontiguousarray(Wk[ms, :].T).astype(bf16),
            "wvt": np.ascontiguousarray(Wv[ms, :].T).astype(bf16),
            "wot": np.ascontiguousarray(Wo[:, ms].T).astype(bf16),
            "mask": mask,
            "ident": identm,
            "ind": indm,
        })
    return in_maps


def _assemble(results):
    out = np.zeros((B, S, D), dtype=np.float32)
    for c in range(N_CORES):
        out[c // 4] += results[c]["out"].T.astype(np.float32)
    return out


def kernel(x, Wq, bq, Wk, bk, Wv, bv, Wo, bo, **_run_kwargs):
    x = np.asarray(x, dtype=np.float32)
    in_maps = _make_in_maps(x, np.asarray(Wq), np.asarray(Wk),
                            np.asarray(Wv), np.asarray(Wo))
    nc = _get_nc()
    res = run_bass_kernel_spmd(nc, in_maps, core_ids=list(range(N_CORES)),
                               **_run_kwargs)
    out = _assemble(res.results)
    # biases are zero in this problem's setup; add anyway for faithfulness
    out += np.asarray(bo, dtype=np.float32)[None, None, :]
    return out


def kernel_traced(x, Wq, bq, Wk, bk, Wv, bv, Wo, bo, trace_cores=None):
    """test.py helper: returns (output, BassKernelResults with exec_time)."""
    x = np.asarray(x, dtype=np.float32)
    in_maps = _make_in_maps(x, np.asarray(Wq), np.asarray(Wk),
                            np.asarray(Wv), np.asarray(Wo))
    nc = _get_nc()
    res = run_bass_kernel_spmd(nc, in_maps, core_ids=list(range(N_CORES)),
                               trace=True, trace_cores=trace_cores)
    out = _assemble(res.results)
    out += np.asarray(bo, dtype=np.float32)[None, None, :]
    return out, res



# revision 17
# speedup vs baseline: 1.2376x; 1.2376x over previous
"""Causal self-attention (B=2, S=2048, D=1024, H=16) on 8 TRN2 NeuronCores.

Sharding: core c -> batch b = c//4, head group g = c%4 (heads 4g..4g+4,
i.e. 256 of the 1024 projection dims). No collectives: each core emits a
transposed partial output out.T = (ans_local @ Wo_cols.T).T of shape
[1024, 2048]; the host transposes and sums the 4 partials per batch.

v2 scheduling notes (vs the first working version):
  - All HBM layouts are per-partition contiguous so every input DMA is a
    128-descriptor transfer; inputs are split across the two HWDGE rings
    (sync: wq/xq0/xq2/xq3+outs, scalar: consts/wk/wv/xq1/wot) with the
    scalar ring's issues done before the first exp needs the engine.
  - Unit order (0,0),(1,0),(0,1),(1,1),... and the Wo projection for
    chunk qc enters the filler stream as soon as both pairs' qc columns
    are normalized, so the tail is one Wo unit instead of four.
  - Fillers (QKV projection + Wo) are emitted in ~cycle-budgeted slices
    between attention k-tiles so the PE instruction stream stays dense:
    a matmul whose semaphore resolves before the previous one drains
    streams back-to-back (213ns/512col) instead of paying the ~166ns
    isolated-dispatch refill, and HAM stays at K=8/8.
  - ScalarE runs (almost) only the exps; PSUM->SBUF staging runs on DVE.
  - Softmax denominators: V is augmented with a ones-column block so PV
    also produces denominator rows for free; 1/den via DVE
    reciprocal_approx_fast straight out of PSUM; one indicator-matmul
    broadcast + one tensor_mul per unit normalizes both heads at once.
"""
import sys

if "/opt/trn_rl_repo" not in sys.path:
    sys.path.insert(0, "/opt/trn_rl_repo")

import numpy as np
import ml_dtypes

import concourse.bacc as bacc
import concourse.tile as tile
from concourse import mybir
from concourse.bass_utils import run_bass_kernel_spmd

N_CORES = 8
B, S, D, H = 2, 2048, 1024, 16
HD = D // H          # 64
HEADS_PER_CORE = 4   # 2 pairs
MLOC = HEADS_PER_CORE * HD  # 256 local projection dims per core
QC = 512             # q chunk width
NQC = S // QC        # 4
NKT = S // 128       # 16 k tiles of 128
KT_PER_QC = QC // 128  # 4
TOTAL_KT = 2 * sum(KT_PER_QC * (qc + 1) for qc in range(NQC))  # 80

BF16 = mybir.dt.bfloat16
F32 = mybir.dt.float32
AF = mybir.ActivationFunctionType

_CACHED_NC = None
_DEBUG = False


def _build_nc():
    nc = bacc.Bacc("TRN2", target_bir_lowering=False, debug=False,
                   enable_asserts=False, num_devices=N_CORES)

    # HBM layouts: everything per-partition contiguous (see _make_in_maps).
    xq_d = nc.dram_tensor("xq", [128, NQC, 8, QC], BF16,
                          kind="ExternalInput").ap()
    wq_d = nc.dram_tensor("wq", [128, 8, MLOC], BF16,
                          kind="ExternalInput").ap()
    wk_d = nc.dram_tensor("wk", [128, 8, MLOC], BF16,
                          kind="ExternalInput").ap()
    wv_d = nc.dram_tensor("wv", [128, 8, MLOC], BF16,
                          kind="ExternalInput").ap()
    wot_d = nc.dram_tensor("wot", [128, 2, D], BF16,
                           kind="ExternalInput").ap()
    # consts: cols 0:256 mask2 (two causal -3e8 masks), 256:384 identity,
    # 384:512 pair-broadcast indicator (rows 0/64 -> cols 0:64 one-hot of
    # local row 0, rows 32/96 -> cols 64:128 one-hot of local row 32)
    cst_d = nc.dram_tensor("cst", [128, 512], BF16, kind="ExternalInput").ap()
    out_d = nc.dram_tensor("out", [NQC, 8, 128, QC], BF16,
                           kind="ExternalOutput").ap()
    if _DEBUG:
        dbg = {
            "dqt0": nc.dram_tensor("dqt0", [128, S], BF16,
                                   kind="ExternalOutput").ap(),
            "dkt0": nc.dram_tensor("dkt0", [128, S], BF16,
                                   kind="ExternalOutput").ap(),
            "dv": nc.dram_tensor("dv", [128, NKT, HEADS_PER_CORE, 128], BF16,
                                 kind="ExternalOutput").ap(),
            "dsrows": nc.dram_tensor("dsrows", [128, 2 * S], F32,
                                     kind="ExternalOutput").ap(),
            "dat0": nc.dram_tensor("dat0", [128, S], BF16,
                                   kind="ExternalOutput").ap(),
            "dat1": nc.dram_tensor("dat1", [128, S], BF16,
                                   kind="ExternalOutput").ap(),
        }

    with tile.TileContext(nc) as tc:
        with tc.tile_pool(name="const", bufs=1) as cpool, \
             tc.tile_pool(name="qkv_sb", bufs=1) as qkvpool, \
             tc.tile_pool(name="pt", bufs=4) as ptpool, \
             tc.tile_pool(name="ostage", bufs=8) as opool, \
             tc.tile_pool(name="au", bufs=2) as aupool, \
             tc.tile_pool(name="ps_big", bufs=2, space="PSUM") as psb, \
             tc.tile_pool(name="ps_ot", bufs=2, space="PSUM") as psot, \
             tc.tile_pool(name="ps_fill", bufs=2, space="PSUM") as psf:

            # ---- SBUF tiles ----
            cst = cpool.tile([128, 512], BF16)
            mask2 = cst[:, 0:256]
            ident = cst[:, 256:384]
            xt = cpool.tile([128, NQC, 8, QC], BF16)
            wq = cpool.tile([128, 8, MLOC], BF16)
            wk = cpool.tile([128, 8, MLOC], BF16)
            wv = cpool.tile([128, 8, MLOC], BF16)
            wot = cpool.tile([128, 2, D], BF16)
            QT = [qkvpool.tile([128, S], BF16, tag=f"qt{p}", name=f"qt{p}")
                  for p in range(2)]
            KT = [qkvpool.tile([128, S], BF16, tag=f"kt{p}", name=f"ktile{p}")
                  for p in range(2)]
            V = qkvpool.tile([128, NKT, HEADS_PER_CORE, 128], BF16)
            ansT = [qkvpool.tile([128, S], BF16, tag=f"at{p}", name=f"at{p}")
                    for p in range(2)]
            # denominator staging: rows 0 (even head) / 32 (odd head) only —
            # reciprocal_approx_fast (custom DVE op) misbehaves at partition
            # bases >= 64, so pairs are separated by column offset p*S.
            srows = cpool.tile([128, 2 * S], F32, name="srows")
            rq = cpool.tile([128, 2 * S], F32, name="rq")
            rq16 = cpool.tile([128, 2 * S], BF16, name="rq16")

            # ---- input DMA schedule ----
            # scalar ring first (it must go idle before the first exp):
            nc.scalar.dma_start(cst[:], cst_d)
            nc.scalar.dma_start(wk[:], wk_d)
            nc.scalar.dma_start(wv[:], wv_d)
            nc.scalar.dma_start(xt[:, 1], xq_d[:, 1])
            nc.scalar.dma_start(wot[:], wot_d)
            # sync ring:
            nc.sync.dma_start(wq[:], wq_d)
            nc.sync.dma_start(xt[:, 0, 0:4], xq_d[:, 0, 0:4])
            nc.sync.dma_start(xt[:, 0, 4:8], xq_d[:, 0, 4:8])
            nc.sync.dma_start(xt[:, 2], xq_d[:, 2])
            nc.sync.dma_start(xt[:, 3], xq_d[:, 3])

            # one-time fills on the idle Pool engine
            nc.gpsimd.memset(V[:, :, :, HD:], 1.0)
            nc.gpsimd.memset(srows[:], 1.0)

            # ---- HAM warm-up: cheap matmuls as soon as the consts land ----
            for _ in range(16):
                w = psf.tile([128, QC], F32, tag="fill", name="warm")
                nc.tensor.matmul(w[:, 0:128], ident, ident,
                                 start=True, stop=True)

            # ---- filler machinery ----
            # Generators yield their approximate PE cycle cost per slice;
            # pump() interleaves them between attention k-tiles.
            fill_q = []            # [(label, gen)]
            done_units = set()
            state = {"fill_cycles": 0, "kt_left": TOTAL_KT}

            def fill_append(label, gen, cycles):
                fill_q.append((label, gen))
                state["fill_cycles"] += cycles

            def pump(budget):
                while budget > 0 and fill_q:
                    label, gen = fill_q[0]
                    try:
                        c = next(gen)
                        budget -= c
                        state["fill_cycles"] -= c
                    except StopIteration:
                        done_units.add(label)
                        fill_q.pop(0)

            def require(labels):
                for lab in labels:
                    while fill_q and lab not in done_units:
                        cur_lab, gen = fill_q[0]
                        for c in gen:
                            state["fill_cycles"] -= c
                        done_units.add(cur_lab)
                        fill_q.pop(0)
                        if cur_lab == lab:
                            break

            def q_gen(p, qc, w_t, dst):
                ps = psf.tile([128, QC], F32, tag="fill", name="ps_qk")
                for dt in range(8):
                    nc.tensor.matmul(
                        ps[:], w_t[:, dt, 128 * p:128 * (p + 1)],
                        xt[:, qc, dt, :], start=(dt == 0), stop=(dt == 7))
                    yield 512
                nc.vector.tensor_copy(dst[:, QC * qc:QC * (qc + 1)], ps[:])

            def v_gen(st):
                qcv, lv = divmod(st, KT_PER_QC)
                ps = psf.tile([128, QC], F32, tag="fill", name="ps_v")
                for dt in range(8):
                    nc.tensor.matmul(
                        ps[:, 0:MLOC],
                        xt[:, qcv, dt, 128 * lv:128 * (lv + 1)],
                        wv[:, dt, :], start=(dt == 0), stop=(dt == 7))
                    yield 256
                nc.vector.tensor_copy(
                    V[:, st, :, 0:HD],
                    ps[:, 0:MLOC].rearrange("p (h c) -> p h c",
                                            h=HEADS_PER_CORE))

            def wo_gen(qc):
                for nt in range(8):
                    po = psf.tile([128, QC], F32, tag="fill", name="po")
                    for mt in range(2):
                        nc.tensor.matmul(
                            po[:], wot[:, mt, 128 * nt:128 * (nt + 1)],
                            ansT[mt][:, QC * qc:QC * (qc + 1)],
                            start=(mt == 0), stop=(mt == 1))
                    ob = opool.tile([128, QC], BF16, tag="ob", name="ob")
                    if nt % 2 == 0:
                        nc.vector.tensor_copy(ob[:], po[:])
                    else:
                        nc.scalar.copy(ob[:], po[:])
                    nc.sync.dma_start(out_d[qc, nt], ob[:])
                    yield 1024

            # ---- per-unit normalization ----
            deferred = []
            wo_ready = []

            def make_finisher(p, qc, au):
                cols = slice(p * S + QC * qc, p * S + QC * (qc + 1))
                acols = slice(QC * qc, QC * (qc + 1))

                def fin():
                    nc.vector.reciprocal_approx_fast(rq[0:33, cols],
                                                     srows[0:33, cols])
                    nc.vector.tensor_copy(rq16[0:33, cols], rq[0:33, cols])
                    bc = psf.tile([128, QC], F32, tag="fill", name="bc")
                    nc.tensor.matmul(bc[:], cst[0:33, 384:512],
                                     rq16[0:33, cols],
                                     start=True, stop=True)
                    nc.vector.tensor_mul(ansT[p][:, acols], au[:], bc[:])
                    if p == 1:
                        wo_ready.append(qc)
                return fin

            def attn(p, qc):
                nkt = KT_PER_QC * (qc + 1)
                ot_a = psot.tile([128, QC], F32, tag="ot", name="ot_a")
                ot_b = psot.tile([128, QC], F32, tag="ot", name="ot_b")
                for kt in range(nkt):
                    r = kt - KT_PER_QC * qc
                    col0 = 128 * r if r >= 0 else 0
                    diag = r >= 0
                    stp = psb.tile([128, 2, QC], F32, tag="big", name="stp")
                    pt = ptpool.tile([128, 2, QC], BF16, tag="pt", name="pt")
                    nc.tensor.matmul(
                        stp[:, 0, col0:QC],
                        KT[p][0:64, 128 * kt:128 * (kt + 1)],
                        QT[p][0:64, QC * qc + col0:QC * (qc + 1)],
                        start=True, stop=not diag)
                    if diag:
                        nc.tensor.matmul(
                            stp[:, 0, col0:col0 + 128], ident,
                            mask2[:, 0:128], start=False, stop=True)
                    nc.tensor.matmul(
                        stp[:, 1, col0:QC],
                        KT[p][64:128, 128 * kt:128 * (kt + 1)],
                        QT[p][64:128, QC * qc + col0:QC * (qc + 1)],
                        start=True, stop=not diag)
                    if diag:
                        nc.tensor.matmul(
                            stp[:, 1, col0:col0 + 128], ident,
                            mask2[:, 128:256], start=False, stop=True)
                    if r > 0:
                        nc.scalar.activation(pt[:, :, col0:], stp[:, :, col0:],
                                             AF.Exp, scale=0.125)
                    else:
                        nc.scalar.activation(pt[:], stp[:], AF.Exp,
                                             scale=0.125)
                    nc.tensor.matmul(
                        ot_a[:, col0:QC], V[:, kt, 2 * p, :],
                        pt[:, 0, col0:QC],
                        start=(kt == 0), stop=(kt == nkt - 1))
                    nc.tensor.matmul(
                        ot_b[:, col0:QC], V[:, kt, 2 * p + 1, :],
                        pt[:, 1, col0:QC],
                        start=(kt == 0), stop=(kt == nkt - 1))
                    if kt == 1:
                        while deferred:
                            deferred.pop(0)()
                        while wo_ready:
                            wqc = wo_ready.pop(0)
                            fill_append(("wo", wqc), wo_gen(wqc), 8192)
                    pump(state["fill_cycles"] // max(state["kt_left"], 1))
                    state["kt_left"] -= 1
                # unit end: stage unnormalized O.T + denominator rows
                cols = slice(p * S + QC * qc, p * S + QC * (qc + 1))
                au = aupool.tile([128, QC], BF16, tag="au", name="au")
                nc.vector.tensor_copy(au[0:64, :], ot_a[0:64, :])
                nc.vector.tensor_copy(au[64:128, :], ot_b[0:64, :])
                nc.vector.tensor_copy(srows[0:1, cols], ot_a[64:65, :])
                nc.vector.tensor_copy(srows[32:33, cols], ot_b[64:65, :])
                deferred.append(make_finisher(p, qc, au))

            # ---- pre-phase: first QKV tiles (DMA-paced) ----
            for g in q_gen(0, 0, wq, QT[0]):
                pass
            for g in q_gen(0, 0, wk, KT[0]):
                pass
            for st in range(KT_PER_QC):
                for g in v_gen(st):
                    pass

            # ---- filler supply ----
            fill_append(("q", 1, 0), q_gen(1, 0, wq, QT[1]), 4096)
            fill_append(("k", 1, 0), q_gen(1, 0, wk, KT[1]), 4096)
            for st in range(4, 8):
                fill_append(("v", st), v_gen(st), 2048)
            fill_append(("q", 0, 1), q_gen(0, 1, wq, QT[0]), 4096)
            fill_append(("k", 0, 1), q_gen(0, 1, wk, KT[0]), 4096)
            fill_append(("q", 1, 1), q_gen(1, 1, wq, QT[1]), 4096)
            fill_append(("k", 1, 1), q_gen(1, 1, wk, KT[1]), 4096)
            for st in range(8, 12):
                fill_append(("v", st), v_gen(st), 2048)
            fill_append(("q", 0, 2), q_gen(0, 2, wq, QT[0]), 4096)
            fill_append(("k", 0, 2), q_gen(0, 2, wk, KT[0]), 4096)
            fill_append(("q", 1, 2), q_gen(1, 2, wq, QT[1]), 4096)
            fill_append(("k", 1, 2), q_gen(1, 2, wk, KT[1]), 4096)
            for st in range(12, 16):
                fill_append(("v", st), v_gen(st), 2048)
            fill_append(("q", 0, 3), q_gen(0, 3, wq, QT[0]), 4096)
            fill_append(("k", 0, 3), q_gen(0, 3, wk, KT[0]), 4096)
            fill_append(("q", 1, 3), q_gen(1, 3, wq, QT[1]), 4096)
            fill_append(("k", 1, 3), q_gen(1, 3, wk, KT[1]), 4096)

            reqs = {
                (1, 0): [("q", 1, 0), ("k", 1, 0)],
                (0, 1): [("v", 7), ("q", 0, 1), ("k", 0, 1)],
                (1, 1): [("q", 1, 1), ("k", 1, 1)],
                (0, 2): [("v", 11), ("q", 0, 2), ("k", 0, 2)],
                (1, 2): [("q", 1, 2), ("k", 1, 2)],
                (0, 3): [("v", 15), ("q", 0, 3), ("k", 0, 3)],
                (1, 3): [("q", 1, 3), ("k", 1, 3)],
            }
            for qc in range(NQC):
                for p in range(2):
                    require(reqs.get((p, qc), []))
                    attn(p, qc)
            while deferred:
                deferred.pop(0)()
            # drain leftover fillers (normally empty except wo(3))
            while wo_ready:
                wqc = wo_ready.pop(0)
                fill_append(("wo", wqc), wo_gen(wqc), 8192)
            pump(1 << 30)
            if _DEBUG:
                nc.sync.dma_start(dbg["dqt0"], QT[0][:])
                nc.sync.dma_start(dbg["dkt0"], KT[0][:])
                nc.sync.dma_start(dbg["dv"], V[:])
                nc.sync.dma_start(dbg["dsrows"], srows[:])
                nc.sync.dma_start(dbg["dat0"], ansT[0][:])
                nc.sync.dma_start(dbg["dat1"], ansT[1][:])

    nc.compile()
    return nc


def _get_nc():
    global _CACHED_NC
    if _CACHED_NC is None:
        _CACHED_NC = _build_nc()
    return _CACHED_NC


def _make_in_maps(x, Wq, Wk, Wv, Wo):
    bf16 = ml_dtypes.bfloat16
    mask = np.where(np.arange(128)[:, None] > np.arange(128)[None, :],
                    np.float32(-3e8), np.float32(0.0)).astype(bf16)
    cst = np.zeros((128, 512), dtype=bf16)
    cst[:, 0:128] = mask
    cst[:, 128:256] = mask
    cst[:, 256:384] = np.eye(128, dtype=bf16)
    ind2 = np.zeros((128, 128), dtype=bf16)
    for r0 in (0, 64):
        ind2[r0, 0:64] = 1.0
        ind2[r0 + 32, 64:128] = 1.0
    cst[:, 384:512] = ind2

    def wlayout(Wslice):
        # [256, 1024] slice -> [128, 8, 256]: w[p, dt, m] = Wslice[m, 128dt+p]
        return np.ascontiguousarray(
            Wslice.T.reshape(8, 128, MLOC).transpose(1, 0, 2)).astype(bf16)

    in_maps = []
    for c in range(N_CORES):
        b, g = divmod(c, 4)
        ms = slice(MLOC * g, MLOC * (g + 1))
        xb = np.asarray(x[b])  # [S, D]
        xq = np.ascontiguousarray(
            xb.reshape(NQC, QC, 8, 128).transpose(3, 0, 2, 1)).astype(bf16)
        WoS = np.asarray(Wo)[:, ms]  # [1024, 256]
        wot = np.ascontiguousarray(
            WoS.T.reshape(2, 128, D).transpose(1, 0, 2)).astype(bf16)
        in_maps.append({
            "xq": xq,
            "wq": wlayout(np.asarray(Wq)[ms, :]),
            "wk": wlayout(np.asarray(Wk)[ms, :]),
            "wv": wlayout(np.asarray(Wv)[ms, :]),
            "wot": wot,
            "cst": cst,
        })
    return in_maps


def _assemble(results):
    out = np.zeros((B, S, D), dtype=np.float32)
    for c in range(N_CORES):
        blk = results[c]["out"].astype(np.float32)  # [NQC, 8, 128, QC]
        # out.T[128nt+p, 512qc+s] = blk[qc, nt, p, s]
        outT = blk.transpose(1, 2, 0, 3).reshape(D, S)
        out[c // 4] += outT.T
    return out


def kernel(x, Wq, bq, Wk, bk, Wv, bv, Wo, bo, **_run_kwargs):
    x = np.asarray(x, dtype=np.float32)
    in_maps = _make_in_maps(x, np.asarray(Wq), np.asarray(Wk),
                            np.asarray(Wv), np.asarray(Wo))
    nc = _get_nc()
    res = run_bass_kernel_spmd(nc, in_maps, core_ids=list(range(N_CORES)),
                               **_run_kwargs)
    out = _assemble(res.results)
    # biases are zero in this problem's setup; add anyway for faithfulness
    out += np.asarray(bo, dtype=np.float32)[None, None, :]
    return out


def kernel_traced(x, Wq, bq, Wk, bk, Wv, bv, Wo, bo, trace_cores=None):
    """test.py helper: returns (output, BassKernelResults with exec_time)."""
    x = np.asarray(x, dtype=np.float32)
    in_maps = _make_in_maps(x, np.asarray(Wq), np.asarray(Wk),
                            np.asarray(Wv), np.asarray(Wo))
    nc = _get_nc()
    res = run_bass_kernel_spmd(nc, in_maps, core_ids=list(range(N_CORES)),
                               trace=True, trace_cores=trace_cores)
    out = _assemble(res.results)
    out += np.asarray(bo, dtype=np.float32)[None, None, :]
    return out, res


# revision 26
# speedup vs baseline: 1.3069x; 1.0560x over previous
"""Causal self-attention (B=2, S=2048, D=1024, H=16) on 8 TRN2 NeuronCores.

Sharding: core c -> batch b = c//4, head group g = c%4 (heads 4g..4g+4,
i.e. 256 of the 1024 projection dims). No collectives: each core emits a
transposed partial output out.T = (ans_local @ Wo_cols.T).T of shape
[1024, 2048]; the host transposes and sums the 4 partials per batch.

v2 scheduling notes (vs the first working version):
  - All HBM layouts are per-partition contiguous so every input DMA is a
    128-descriptor transfer; inputs are split across the two HWDGE rings
    (sync: wq/xq0/xq2/xq3+outs, scalar: consts/wk/wv/xq1/wot) with the
    scalar ring's issues done before the first exp needs the engine.
  - Unit order (0,0),(1,0),(0,1),(1,1),... and the Wo projection for
    chunk qc enters the filler stream as soon as both pairs' qc columns
    are normalized, so the tail is one Wo unit instead of four.
  - Fillers (QKV projection + Wo) are emitted in ~cycle-budgeted slices
    between attention k-tiles so the PE instruction stream stays dense:
    a matmul whose semaphore resolves before the previous one drains
    streams back-to-back (213ns/512col) instead of paying the ~166ns
    isolated-dispatch refill, and HAM stays at K=8/8.
  - ScalarE runs (almost) only the exps; PSUM->SBUF staging runs on DVE.
  - Softmax denominators: V is augmented with a ones-column block so PV
    also produces denominator rows for free; 1/den via DVE
    reciprocal_approx_fast straight out of PSUM; one indicator-matmul
    broadcast + one tensor_mul per unit normalizes both heads at once.
"""
import sys

if "/opt/trn_rl_repo" not in sys.path:
    sys.path.insert(0, "/opt/trn_rl_repo")

import numpy as np
import ml_dtypes

import concourse.bacc as bacc
import concourse.tile as tile
from concourse import mybir
from concourse.bass_utils import run_bass_kernel_spmd

N_CORES = 8
B, S, D, H = 2, 2048, 1024, 16
HD = D // H          # 64
HEADS_PER_CORE = 4   # 2 pairs
MLOC = HEADS_PER_CORE * HD  # 256 local projection dims per core
QC = 512             # q chunk width
NQC = S // QC        # 4
NKT = S // 128       # 16 k tiles of 128
KT_PER_QC = QC // 128  # 4
TOTAL_KT = 2 * sum(KT_PER_QC * (qc + 1) for qc in range(NQC))  # 80

BF16 = mybir.dt.bfloat16
F32 = mybir.dt.float32
AF = mybir.ActivationFunctionType

_CACHED_NC = None
_DEBUG = False


def _build_nc():
    nc = bacc.Bacc("TRN2", target_bir_lowering=False, debug=False,
                   enable_asserts=False, num_devices=N_CORES)

    # HBM layouts: everything per-partition contiguous (see _make_in_maps).
    xq_d = nc.dram_tensor("xq", [128, NQC, 8, QC], BF16,
                          kind="ExternalInput").ap()
    wq_d = nc.dram_tensor("wq", [128, 8, MLOC], BF16,
                          kind="ExternalInput").ap()
    wk_d = nc.dram_tensor("wk", [128, 8, MLOC], BF16,
                          kind="ExternalInput").ap()
    wv_d = nc.dram_tensor("wv", [128, 8, MLOC], BF16,
                          kind="ExternalInput").ap()
    wot_d = nc.dram_tensor("wot", [128, 2, D], BF16,
                           kind="ExternalInput").ap()
    # consts: cols 0:256 two copies of the lower-triangular 0/1 mask (for
    # post-exp zeroing of both heads' diagonal blocks in one op), 256:384
    # pair-broadcast indicator (row 0 -> cols 0:64 one-hot of local row 0,
    # row 32 -> cols 64:128 one-hot of local row 32)
    cst_d = nc.dram_tensor("cst", [128, 384], BF16, kind="ExternalInput").ap()
    out_d = nc.dram_tensor("out", [NQC, 8, 128, QC], BF16,
                           kind="ExternalOutput").ap()
    if _DEBUG:
        dbg = {
            "dqt0": nc.dram_tensor("dqt0", [128, S], BF16,
                                   kind="ExternalOutput").ap(),
            "dkt0": nc.dram_tensor("dkt0", [128, S], BF16,
                                   kind="ExternalOutput").ap(),
            "dv": nc.dram_tensor("dv", [128, NKT, HEADS_PER_CORE, 128], BF16,
                                 kind="ExternalOutput").ap(),
            "dsrows": nc.dram_tensor("dsrows", [128, 2 * S], F32,
                                     kind="ExternalOutput").ap(),
            "dat0": nc.dram_tensor("dat0", [128, S], BF16,
                                   kind="ExternalOutput").ap(),
            "dat1": nc.dram_tensor("dat1", [128, S], BF16,
                                   kind="ExternalOutput").ap(),
        }

    with tile.TileContext(nc) as tc:
        with tc.tile_pool(name="const", bufs=1) as cpool, \
             tc.tile_pool(name="qkv_sb", bufs=1) as qkvpool, \
             tc.tile_pool(name="pt", bufs=4) as ptpool, \
             tc.tile_pool(name="ostage", bufs=8) as opool, \
             tc.tile_pool(name="au", bufs=2) as aupool, \
             tc.tile_pool(name="ps_big", bufs=2, space="PSUM") as psb, \
             tc.tile_pool(name="ps_ot", bufs=2, space="PSUM") as psot, \
             tc.tile_pool(name="ps_fill", bufs=2, space="PSUM") as psf:

            # ---- SBUF tiles ----
            cst = cpool.tile([128, 384], BF16)
            tril2 = cst[:, 0:256].rearrange("p (h c) -> p h c", h=2)
            trilm = cst[:, 0:128]
            xt = cpool.tile([128, NQC, 8, QC], BF16)
            wq = cpool.tile([128, 8, MLOC], BF16)
            wk = cpool.tile([128, 8, MLOC], BF16)
            wv = cpool.tile([128, 8, MLOC], BF16)
            wot = cpool.tile([128, 2, D], BF16)
            QT = [qkvpool.tile([128, S], BF16, tag=f"qt{p}", name=f"qt{p}")
                  for p in range(2)]
            KT = [qkvpool.tile([128, S], BF16, tag=f"kt{p}", name=f"ktile{p}")
                  for p in range(2)]
            V = qkvpool.tile([128, NKT, HEADS_PER_CORE, 128], BF16)
            ansT = [qkvpool.tile([128, S], BF16, tag=f"at{p}", name=f"at{p}")
                    for p in range(2)]
            # denominator staging: rows 0 (even head) / 32 (odd head) only —
            # reciprocal_approx_fast (custom DVE op) misbehaves at partition
            # bases >= 64, so pairs are separated by column offset p*S.
            srows = cpool.tile([128, 2 * S], F32, name="srows")
            rq = cpool.tile([128, 2 * S], F32, name="rq")
            rq16 = cpool.tile([128, 2 * S], BF16, name="rq16")

            # ---- input DMA schedule ----
            # scalar ring first (it must go idle before the first exp):
            nc.scalar.dma_start(cst[:], cst_d)
            nc.scalar.dma_start(wk[:, 0:4], wk_d[:, 0:4])
            nc.scalar.dma_start(wk[:, 4:8], wk_d[:, 4:8])
            nc.scalar.dma_start(wv[:], wv_d)
            nc.scalar.dma_start(xt[:, 1], xq_d[:, 1])
            nc.scalar.dma_start(wot[:], wot_d)
            # sync ring:
            nc.sync.dma_start(wq[:, 0:4], wq_d[:, 0:4])
            nc.sync.dma_start(wq[:, 4:8], wq_d[:, 4:8])
            nc.sync.dma_start(xt[:, 0, 0:4], xq_d[:, 0, 0:4])
            nc.sync.dma_start(xt[:, 0, 4:8], xq_d[:, 0, 4:8])
            nc.sync.dma_start(xt[:, 2], xq_d[:, 2])
            nc.sync.dma_start(xt[:, 3], xq_d[:, 3])

            # one-time fills on the idle Pool engine
            nc.gpsimd.memset(V[:, :, :, HD:], 1.0)
            nc.gpsimd.memset(srows[:], 1.0)

            # ---- HAM warm-up: cheap matmuls as soon as the consts land ----
            for _ in range(16):
                w = psf.tile([128, QC], F32, tag="fill", name="warm")
                nc.tensor.matmul(w[:, 0:128], trilm, trilm,
                                 start=True, stop=True)

            # ---- filler machinery ----
            # Generators yield their approximate PE cycle cost per slice;
            # pump() interleaves them between attention k-tiles.
            fill_q = []            # [(label, gen)]
            done_units = set()
            state = {"fill_cycles": 0, "kt_left": TOTAL_KT}

            def fill_append(label, gen, cycles):
                fill_q.append((label, gen))
                state["fill_cycles"] += cycles

            def pump(budget):
                while budget > 0 and fill_q:
                    label, gen = fill_q[0]
                    try:
                        c = next(gen)
                        budget -= c
                        state["fill_cycles"] -= c
                    except StopIteration:
                        done_units.add(label)
                        fill_q.pop(0)

            def require(labels):
                for lab in labels:
                    while fill_q and lab not in done_units:
                        cur_lab, gen = fill_q[0]
                        for c in gen:
                            state["fill_cycles"] -= c
                        done_units.add(cur_lab)
                        fill_q.pop(0)
                        if cur_lab == lab:
                            break

            def q_gen(p, qc, w_t, dst):
                ps = psf.tile([128, QC], F32, tag="fill", name="ps_qk")
                for dt in range(8):
                    nc.tensor.matmul(
                        ps[:], w_t[:, dt, 128 * p:128 * (p + 1)],
                        xt[:, qc, dt, :], start=(dt == 0), stop=(dt == 7))
                    yield 512
                nc.vector.tensor_copy(dst[:, QC * qc:QC * (qc + 1)], ps[:])

            def v_gen(st):
                qcv, lv = divmod(st, KT_PER_QC)
                ps = psf.tile([128, QC], F32, tag="fill", name="ps_v")
                for dt in range(8):
                    nc.tensor.matmul(
                        ps[:, 0:MLOC],
                        xt[:, qcv, dt, 128 * lv:128 * (lv + 1)],
                        wv[:, dt, :], start=(dt == 0), stop=(dt == 7))
                    yield 256
                nc.vector.tensor_copy(
                    V[:, st, :, 0:HD],
                    ps[:, 0:MLOC].rearrange("p (h c) -> p h c",
                                            h=HEADS_PER_CORE))

            def wo_gen(qc, dual_dma=False):
                for nt in range(8):
                    po = psf.tile([128, QC], F32, tag="fill", name="po")
                    for mt in range(2):
                        nc.tensor.matmul(
                            po[:], wot[:, mt, 128 * nt:128 * (nt + 1)],
                            ansT[mt][:, QC * qc:QC * (qc + 1)],
                            start=(mt == 0), stop=(mt == 1))
                    ob = opool.tile([128, QC], BF16, tag="ob", name="ob")
                    if nt % 2 == 0:
                        nc.vector.tensor_copy(ob[:], po[:])
                    else:
                        nc.scalar.copy(ob[:], po[:])
                    eng = nc.scalar if (dual_dma and nt % 2 == 1) else nc.sync
                    eng.dma_start(out_d[qc, nt], ob[:])
                    yield 1024

            # ---- per-unit normalization ----
            deferred = []
            wo_ready = []

            def make_finisher(p, qc, au):
                cols = slice(p * S + QC * qc, p * S + QC * (qc + 1))
                acols = slice(QC * qc, QC * (qc + 1))

                def fin():
                    nc.vector.reciprocal_approx_fast(rq[0:33, cols],
                                                     srows[0:33, cols])
                    nc.vector.tensor_copy(rq16[0:33, cols], rq[0:33, cols])
                    bc = psf.tile([128, QC], F32, tag="fill", name="bc")
                    nc.tensor.matmul(bc[:], cst[0:33, 256:384],
                                     rq16[0:33, cols],
                                     start=True, stop=True)
                    nc.vector.tensor_mul(ansT[p][:, acols], au[:], bc[:])
                    if p == 1:
                        wo_ready.append(qc)
                return fin

            def attn(p, qc):
                nkt = KT_PER_QC * (qc + 1)
                ot_a = psot.tile([128, QC], F32, tag="ot", name="ot_a")
                ot_b = psot.tile([128, QC], F32, tag="ot", name="ot_b")
                pts = {}

                def emit_scores(kt):
                    r = kt - KT_PER_QC * qc
                    col0 = 128 * r if r >= 0 else 0
                    stp = psb.tile([128, 2, QC], F32, tag="big", name="stp")
                    pt = ptpool.tile([128, 2, QC], BF16, tag="pt", name="pt")
                    nc.tensor.matmul(
                        stp[:, 0, col0:QC],
                        KT[p][0:64, 128 * kt:128 * (kt + 1)],
                        QT[p][0:64, QC * qc + col0:QC * (qc + 1)],
                        start=True, stop=True)
                    nc.tensor.matmul(
                        stp[:, 1, col0:QC],
                        KT[p][64:128, 128 * kt:128 * (kt + 1)],
                        QT[p][64:128, QC * qc + col0:QC * (qc + 1)],
                        start=True, stop=True)
                    if r > 0:
                        nc.scalar.activation(pt[:, :, col0:], stp[:, :, col0:],
                                             AF.Exp, scale=0.125)
                    else:
                        nc.scalar.activation(pt[:], stp[:], AF.Exp,
                                             scale=0.125)
                    if r >= 0:
                        # zero the upper triangle of the diagonal block for
                        # both heads (Pool engine, SBUF-only elementwise)
                        nc.gpsimd.tensor_mul(pt[:, :, col0:col0 + 128],
                                             pt[:, :, col0:col0 + 128],
                                             tril2)
                    pts[kt] = pt

                def emit_pv(kt):
                    r = kt - KT_PER_QC * qc
                    col0 = 128 * r if r >= 0 else 0
                    pt = pts.pop(kt)
                    nc.tensor.matmul(
                        ot_a[:, col0:QC], V[:, kt, 2 * p, :],
                        pt[:, 0, col0:QC],
                        start=(kt == 0), stop=(kt == nkt - 1))
                    nc.tensor.matmul(
                        ot_b[:, col0:QC], V[:, kt, 2 * p + 1, :],
                        pt[:, 1, col0:QC],
                        start=(kt == 0), stop=(kt == nkt - 1))

                emit_scores(0)
                for kt in range(nkt):
                    if kt + 1 < nkt:
                        emit_scores(kt + 1)
                    emit_pv(kt)
                    if kt == 1:
                        while deferred:
                            deferred.pop(0)()
                        while wo_ready:
                            wqc = wo_ready.pop(0)
                            fill_append(("wo", wqc), wo_gen(wqc), 8192)
                    pump(state["fill_cycles"] // max(state["kt_left"], 1))
                    state["kt_left"] -= 1
                # unit end: stage unnormalized O.T + denominator rows
                cols = slice(p * S + QC * qc, p * S + QC * (qc + 1))
                au = aupool.tile([128, QC], BF16, tag="au", name="au")
                nc.vector.tensor_copy(au[0:64, :], ot_a[0:64, :])
                nc.vector.tensor_copy(au[64:128, :], ot_b[0:64, :])
                nc.vector.tensor_copy(srows[0:1, cols], ot_a[64:65, :])
                nc.vector.tensor_copy(srows[32:33, cols], ot_b[64:65, :])
                deferred.append(make_finisher(p, qc, au))

            # ---- pre-phase: first QKV tiles (DMA-paced) ----
            for g in q_gen(0, 0, wq, QT[0]):
                pass
            for g in q_gen(0, 0, wk, KT[0]):
                pass
            for st in range(KT_PER_QC):
                for g in v_gen(st):
                    pass

            # ---- filler supply ----
            fill_append(("q", 1, 0), q_gen(1, 0, wq, QT[1]), 4096)
            fill_append(("k", 1, 0), q_gen(1, 0, wk, KT[1]), 4096)
            for st in range(4, 8):
                fill_append(("v", st), v_gen(st), 2048)
            fill_append(("q", 0, 1), q_gen(0, 1, wq, QT[0]), 4096)
            fill_append(("k", 0, 1), q_gen(0, 1, wk, KT[0]), 4096)
            fill_append(("q", 1, 1), q_gen(1, 1, wq, QT[1]), 4096)
            fill_append(("k", 1, 1), q_gen(1, 1, wk, KT[1]), 4096)
            for st in range(8, 12):
                fill_append(("v", st), v_gen(st), 2048)
            fill_append(("q", 0, 2), q_gen(0, 2, wq, QT[0]), 4096)
            fill_append(("k", 0, 2), q_gen(0, 2, wk, KT[0]), 4096)
            fill_append(("q", 1, 2), q_gen(1, 2, wq, QT[1]), 4096)
            fill_append(("k", 1, 2), q_gen(1, 2, wk, KT[1]), 4096)
            for st in range(12, 16):
                fill_append(("v", st), v_gen(st), 2048)
            fill_append(("q", 0, 3), q_gen(0, 3, wq, QT[0]), 4096)
            fill_append(("k", 0, 3), q_gen(0, 3, wk, KT[0]), 4096)
            fill_append(("q", 1, 3), q_gen(1, 3, wq, QT[1]), 4096)
            fill_append(("k", 1, 3), q_gen(1, 3, wk, KT[1]), 4096)

            reqs = {
                (1, 0): [("q", 1, 0), ("k", 1, 0)],
                (0, 1): [("v", 7), ("q", 0, 1), ("k", 0, 1)],
                (1, 1): [("q", 1, 1), ("k", 1, 1)],
                (0, 2): [("v", 11), ("q", 0, 2), ("k", 0, 2)],
                (1, 2): [("q", 1, 2), ("k", 1, 2)],
                (0, 3): [("v", 15), ("q", 0, 3), ("k", 0, 3)],
                (1, 3): [("q", 1, 3), ("k", 1, 3)],
            }
            for qc in range(NQC):
                for p in range(2):
                    require(reqs.get((p, qc), []))
                    attn(p, qc)
            while deferred:
                deferred.pop(0)()
            # drain leftover fillers (normally empty except wo(3))
            while wo_ready:
                wqc = wo_ready.pop(0)
                fill_append(("wo", wqc), wo_gen(wqc, dual_dma=True), 8192)
            pump(1 << 30)
            if _DEBUG:
                nc.sync.dma_start(dbg["dqt0"], QT[0][:])
                nc.sync.dma_start(dbg["dkt0"], KT[0][:])
                nc.sync.dma_start(dbg["dv"], V[:])
                nc.sync.dma_start(dbg["dsrows"], srows[:])
                nc.sync.dma_start(dbg["dat0"], ansT[0][:])
                nc.sync.dma_start(dbg["dat1"], ansT[1][:])

    nc.compile()
    return nc


def _get_nc():
    global _CACHED_NC
    if _CACHED_NC is None:
        _CACHED_NC = _build_nc()
    return _CACHED_NC


def _make_in_maps(x, Wq, Wk, Wv, Wo):
    bf16 = ml_dtypes.bfloat16
    # validity of the transposed diagonal block: S.T[k, q] valid iff q >= k
    keep = (np.arange(128)[:, None] <= np.arange(128)[None, :]).astype(bf16)
    cst = np.zeros((128, 384), dtype=bf16)
    cst[:, 0:128] = keep
    cst[:, 128:256] = keep
    ind2 = np.zeros((128, 128), dtype=bf16)
    ind2[0, 0:64] = 1.0
    ind2[32, 64:128] = 1.0
    cst[:, 256:384] = ind2

    def wlayout(Wslice):
        # [256, 1024] slice -> [128, 8, 256]: w[p, dt, m] = Wslice[m, 128dt+p]
        return np.ascontiguousarray(
            Wslice.T.reshape(8, 128, MLOC).transpose(1, 0, 2)).astype(bf16)

    in_maps = []
    for c in range(N_CORES):
        b, g = divmod(c, 4)
        ms = slice(MLOC * g, MLOC * (g + 1))
        xb = np.asarray(x[b])  # [S, D]
        xq = np.ascontiguousarray(
            xb.reshape(NQC, QC, 8, 128).transpose(3, 0, 2, 1)).astype(bf16)
        WoS = np.asarray(Wo)[:, ms]  # [1024, 256]
        wot = np.ascontiguousarray(
            WoS.T.reshape(2, 128, D).transpose(1, 0, 2)).astype(bf16)
        in_maps.append({
            "xq": xq,
            "wq": wlayout(np.asarray(Wq)[ms, :]),
            "wk": wlayout(np.asarray(Wk)[ms, :]),
            "wv": wlayout(np.asarray(Wv)[ms, :]),
            "wot": wot,
            "cst": cst,
        })
    return in_maps


def _assemble(results):
    out = np.zeros((B, S, D), dtype=np.float32)
    for c in range(N_CORES):
        blk = results[c]["out"].astype(np.float32)  # [NQC, 8, 128, QC]
        # out.T[128nt+p, 512qc+s] = blk[qc, nt, p, s]
        outT = blk.transpose(1, 2, 0, 3).reshape(D, S)
        out[c // 4] += outT.T
    return out


def kernel(x, Wq, bq, Wk, bk, Wv, bv, Wo, bo, **_run_kwargs):
    x = np.asarray(x, dtype=np.float32)
    in_maps = _make_in_maps(x, np.asarray(Wq), np.asarray(Wk),
                            np.asarray(Wv), np.asarray(Wo))
    nc = _get_nc()
    res = run_bass_kernel_spmd(nc, in_maps, core_ids=list(range(N_CORES)),
                               **_run_kwargs)
    out = _assemble(res.results)
    # biases are zero in this problem's setup; add anyway for faithfulness
    out += np.asarray(bo, dtype=np.float32)[None, None, :]
    return out


def kernel_traced(x, Wq, bq, Wk, bk, Wv, bv, Wo, bo, trace_cores=None):
    """test.py helper: returns (output, BassKernelResults with exec_time)."""
    x = np.asarray(x, dtype=np.float32)
    in_maps = _make_in_maps(x, np.asarray(Wq), np.asarray(Wk),
                            np.asarray(Wv), np.asarray(Wo))
    nc = _get_nc()
    res = run_bass_kernel_spmd(nc, in_maps, core_ids=list(range(N_CORES)),
                               trace=True, trace_cores=trace_cores)
    out = _assemble(res.results)
    out += np.asarray(bo, dtype=np.float32)[None, None, :]
    return out, res
